# revision 38
# baseline (speedup 1.0000x reference)
"""EfficientAttention Trainium2 Bass kernel (v2).

Reference (per token t, H=16 heads, hd=64, d=1024):
  Q = x @ Wq.T ; K = x @ Wk.T ; V = x @ Wv.T
  sK = softmax over heads of K^T : sK[d,h'] = expK[h',d] * rki[d]
  out[h,:] = softmax(Q)[h,:] @ (sK @ V)
           = rqi[h] * sum_h' A[h,h'] V[h',:],  A[h,h'] = sum_d expQ[h,d]*rki[d]*expK[h',d]

Sharding: data-parallel over 16384 tokens, 2048/core, weights replicated.

Per 128-token tile:
  PE : 3 projections (bf16, token-major psum) + per-token mm1 (A^T, ap=16)
       + per-token mm2 (out^T, ap=16).  No PE transposes at all.
  DMA XBAR transpose builds every awkward layout:
       QS/KS slabs [64 d, (t,h)]  <- token-major sqt/expk
       VS2 [16 h', (t,e)]         <- V projection with host-permuted Wv rows
       bt  [128 t, (h,e)]         <- out^T slab (padded 3D out AP)
  ACT: psum evictions (exp for Q/K, copies for V/A/out^T)
  DVE: softmax normalizers, sqt=expQ*rki, final out = bt*rqi (fp32)

x is host-pre-transposed to [1024, 2048] per core so PE never transposes it.
Emission is software-pipelined: iter i runs mm1(i-1), mm2(i-2), proj(i) on PE.
"""

import os
import numpy as np
import ml_dtypes
from contextlib import ExitStack

import concourse.bass as bass
import concourse.mybir as mybir
import concourse.tile as tile
from concourse import bacc
from concourse.bass_utils import run_bass_kernel_spmd

DIMS = 1024
HEADS = 16
HD = 64
N_CORES = 8
B, L = 4, 4096
TOKENS = B * L
TOK_PER_CORE = TOKENS // N_CORES  # 2048
P = 128
N_TILES = TOK_PER_CORE // P       # 16

V_PP = int(os.environ.get("KV_PP", "2"))
V_PA = int(os.environ.get("KV_PA", "2"))
V_OT = int(os.environ.get("KV_OT", "2"))
V_AEV_DVE = int(os.environ.get("KV_AEV_DVE", "1"))
V_VT2_DVE = int(os.environ.get("KV_VT2_DVE", "0"))
FP32 = mybir.dt.float32
BF16 = mybir.dt.bfloat16
FP8 = mybir.dt.float8e4
EXP = mybir.ActivationFunctionType.Exp
COPY = mybir.ActivationFunctionType.Copy
WSCALE = 32.0

_COMPILED = {}
SECTIONS = []  # (instruction counter, label) markers for profiling


def _build_kernel():
    nc = bacc.Bacc("TRN2", target_bir_lowering=False)

    def mark(label):
        SECTIONS.append((int(nc.get_next_instruction_name()[2:]), label))

    x8_in = nc.dram_tensor("x8", [TOK_PER_CORE, DIMS], FP8, kind="ExternalInput")
    xr_in = nc.dram_tensor("xr", [TOK_PER_CORE, DIMS], FP8, kind="ExternalInput")
    w_ins = {}
    for nm in ("k8", "kr", "q8", "qr", "v8", "vr"):
        w_ins[nm] = nc.dram_tensor("w" + nm, [DIMS, DIMS], FP8,
                                   kind="ExternalInput")
    out_d = nc.dram_tensor("out", [TOK_PER_CORE, DIMS], FP32, kind="ExternalOutput")
    ident_in = nc.dram_tensor("ident", [P, P], BF16, kind="ExternalInput")

    with tile.TileContext(nc) as tc, ExitStack() as ctx:
        wpool = ctx.enter_context(tc.tile_pool(name="weights", bufs=1))
        xpool = ctx.enter_context(tc.tile_pool(name="x", bufs=3))
        evpool = ctx.enter_context(tc.tile_pool(name="ev", bufs=3))
        sqpool = ctx.enter_context(tc.tile_pool(name="sq", bufs=3))
        slabpool = ctx.enter_context(tc.tile_pool(name="slab", bufs=3))
        vspool = ctx.enter_context(tc.tile_pool(name="vs", bufs=3))
        aevpool = ctx.enter_context(tc.tile_pool(name="aev", bufs=2))
        otspool = ctx.enter_context(tc.tile_pool(name="ots", bufs=2))
        btpool = ctx.enter_context(tc.tile_pool(name="bt", bufs=2))
        opool = ctx.enter_context(tc.tile_pool(name="outs", bufs=2))
        npool = ctx.enter_context(tc.tile_pool(name="norm", bufs=5))
        # PSUM: proj 3 banks + mm1 3 + mm2 2 = 8
        ps_pp = ctx.enter_context(tc.tile_pool(name="ps_pp", bufs=V_PP, space="PSUM"))
        ps_pa = ctx.enter_context(tc.tile_pool(name="ps_pa", bufs=V_PA, space="PSUM"))
        ps_ot = ctx.enter_context(tc.tile_pool(name="ps_ot", bufs=V_OT, space="PSUM"))
        ps_ep = ctx.enter_context(tc.tile_pool(name="ps_ep", bufs=2, space="PSUM"))

        ident = None

        def load_x(i):
            # host pre-tiled fp8 hi/lo: row it*128+p holds x^T chunk data
            x8t = xpool.tile([P, DIMS], FP8, tag="x8", name=f"x8_{i}")
            nc.gpsimd.dma_start(x8t[:], x8_in[i * P:(i + 1) * P, :])
            xrt = xpool.tile([P, DIMS], FP8, tag="xr", name=f"xr_{i}")
            nc.gpsimd.dma_start(xrt[:], xr_in[i * P:(i + 1) * P, :])
            return (x8t, xrt)

        xts = {0: load_x(0), 1: load_x(1)}

        dscr = ctx.enter_context(tc.tile_pool(name="dscr", bufs=2, space="DRAM"))
        consts = ctx.enter_context(tc.tile_pool(name="consts", bufs=1))
        ident = consts.tile([P, P], BF16)
        nc.sync.dma_start(ident[:], ident_in[:])

        ws = {}
        for name in ("k8", "kr", "q8", "qr", "v8", "vr"):
            w = wpool.tile([P, 8 * DIMS], FP8, tag=f"w{name}")
            for c in range(8):
                nc.sync.dma_start(w[:, c * DIMS:(c + 1) * DIMS],
                                  w_ins[name][c * P:(c + 1) * P, :])
            ws[name] = w

        # cross-iteration state, keyed by tile index
        st = {}

        def emit_mm1(j):
            """A^T[h',(t,h)] for tile j; pa psum evicted to aev slab."""
            mark(f"mm1({j})")
            s = st[j]
            aev = aevpool.tile([HEADS, P * HEADS], BF16, tag="aev",
                               name=f"aev{j}")
            for bk in range(4):
                pa = ps_pa.tile([HEADS, 512], FP32, tag="pa", name=f"pa{j}_{bk}")
                for ts in range(32):
                    t = 32 * bk + ts
                    nc.tensor.matmul(
                        pa[:, ts * HEADS:(ts + 1) * HEADS],
                        lhsT=s["ks"][:, t::P],
                        rhs=s["qs"][:, t::P],
                        start=True, stop=True)
                if V_AEV_DVE:
                    nc.vector.tensor_scalar_mul(
                        aev[:, bk * 512:(bk + 1) * 512], pa[:], 1.0)
                else:
                    nc.scalar.copy(aev[:, bk * 512:(bk + 1) * 512], pa[:])
            s["aev"] = aev

        def emit_mm2_post(j):
            """out^T for tile j, back-transpose, rqi scale, store."""
            mark(f"mm2({j})")
            s = st[j]
            ots = otspool.tile([HD, P * HEADS], BF16, tag="ots", name=f"ots{j}")
            for bk in range(4):
                po = ps_ot.tile([HD, 512], FP32, tag="po", name=f"po{j}_{bk}")
                for ts in range(32):
                    t = 32 * bk + ts
                    nc.tensor.matmul(
                        po[:, ts * HEADS:(ts + 1) * HEADS],
                        lhsT=s["vs2"][:, t * HD:(t + 1) * HD],
                        rhs=s["aev"][:, t * HEADS:(t + 1) * HEADS],
                        start=True, stop=True)
                # evict with (t,h) -> (h,t) column permute: ots[e, h*128+t]
                dst = ots[:].rearrange("p (h t) -> p h t", t=P)[
                    :, :, bk * 32:(bk + 1) * 32]
                srcv = po[:].rearrange("p (t h) -> p h t", h=HEADS)
                nc.scalar.copy(dst, srcv)
            s["ots"] = ots

        def emit_post(j):
            """Deferred by one iter: PE back-transpose, rqi scale, store."""
            mark(f"post({j})")
            s = st[j]
            btp = ps_ep.tile([P, DIMS], BF16, tag="bt", bufs=1,
                             name=f"bt{j}")
            ots = s["ots"]
            for h in range(HEADS):
                nc.tensor.transpose(
                    btp[:, h * HD:(h + 1) * HD],
                    ots[:, h * P:(h + 1) * P], ident[0:HD, 0:HD])
            osb = opool.tile([P, DIMS], FP32, tag="osb", name=f"osb{j}")
            rqi_b = s["rqi"][:].unsqueeze(2).broadcast_to([P, HEADS, HD])
            nc.vector.tensor_mul(
                osb[:].rearrange("p (h e) -> p h e", e=HD),
                btp[:].rearrange("p (h e) -> p h e", e=HD), rqi_b)
            nc.gpsimd.dma_start(out_d[j * P:(j + 1) * P, :], osb[:])

        def proj(i, xpair, pname, dst, func):
            x8t, xrt = xpair
            w8, wr = ws[pname + "8"], ws[pname + "r"]
            x8v = x8t[:].rearrange("p (c t) -> p c t", t=P)
            xrv = xrt[:].rearrange("p (c t) -> p c t", t=P)
            w8v = w8[:].rearrange("p (c f) -> p c f", f=DIMS)
            wrv = wr[:].rearrange("p (c f) -> p c f", f=DIMS)
            for nb in range(2):
                pp = ps_pp.tile([P, 512], FP32, tag="pp",
                                name=f"pp{i}_{pname}{nb}")
                fs = slice(nb * 512, nb * 512 + 512)
                terms = [(x8v, w8v), (xrv, w8v), (x8v, wrv)]
                n_mm = 4 * len(terms)
                k = 0
                for c2 in range(4):
                    cs = slice(2 * c2, 2 * c2 + 2)
                    for xv, wv in terms:
                        nc.tensor.matmul(
                            pp[:],
                            lhsT=xv[:, cs, :],
                            rhs=wv[:, cs, fs],
                            perf_mode=mybir.MatmulPerfMode.DoubleRow,
                            start=(k == 0), stop=(k == n_mm - 1))
                        k += 1
                sl = fs
                if func is None:
                    nc.scalar.activation(dst[:, sl], pp[:], COPY,
                                         scale=1.0 / WSCALE)
                else:
                    nc.scalar.activation(dst[:, sl], pp[:], func,
                                         scale=1.0 / WSCALE)

        def emit_projK(i):
            mark(f"projK({i})")
            s = st.setdefault(i, {})
            expk = evpool.tile([P, DIMS], BF16, tag="expk", name=f"ek{i}")
            s["expk"] = expk
            proj(i, xts[i], "k", expk, EXP)
            # rk[d] = sum_h expK ; rki ; rkib
            t1 = npool.tile([P, 512], BF16, tag="t1")
            nc.vector.tensor_add(t1[:], expk[:, 0:512], expk[:, 512:1024])
            t2 = npool.tile([P, 256], BF16, tag="t2")
            nc.vector.tensor_add(t2[:], t1[:, 0:256], t1[:, 256:512])
            t3 = npool.tile([P, P], BF16, tag="t3")
            nc.vector.tensor_add(t3[:], t2[:, 0:P], t2[:, P:256])
            rk = npool.tile([P, HD], FP32, tag="rk")
            nc.vector.tensor_add(rk[:], t3[:, 0:HD], t3[:, HD:P])
            rkif = npool.tile([P, HD], FP32, tag="rkif")
            nc.vector.reciprocal_approx_fast(rkif[:], rk[:])
            rkib = npool.tile([P, HD], BF16, tag="rkib")
            nc.vector.tensor_scalar_mul(rkib[:], rkif[:], 1.0)
            s["rkib"] = rkib
            s["ks"] = extract(i, expk, "ks")


        def emit_projQ(i):
            mark(f"projQ({i})")
            s = st[i]
            expq = evpool.tile([P, DIMS], BF16, tag="expq", name=f"eq{i}")
            proj(i, xts[i], "q", expq, EXP)
            # sqt = expQ * rki[d]
            sqt = sqpool.tile([P, DIMS], BF16, tag="sqt", name=f"sqt{i}")
            rkib_b = s["rkib"][:].unsqueeze(1).broadcast_to([P, HEADS, HD])
            nc.vector.tensor_mul(
                sqt[:].rearrange("p (h d) -> p h d", d=HD),
                expq[:].rearrange("p (h d) -> p h d", d=HD), rkib_b)
            s["sqt"] = sqt
            s["qs"] = extract(i, sqt, "qs")
            # rq[h] = sum_d expQ via halving adds; rqi fp32 (used 2 iters on)
            eqv = expq[:].rearrange("p (h d) -> p h d", d=HD)
            q1 = npool.tile([P, HEADS * 32], BF16, tag="q1")
            q1v = q1[:].rearrange("p (h d) -> p h d", d=32)
            nc.vector.tensor_add(q1v, eqv[:, :, 0:32], eqv[:, :, 32:64])
            q2 = npool.tile([P, HEADS * 16], BF16, tag="q2")
            q2v = q2[:].rearrange("p (h d) -> p h d", d=16)
            nc.vector.tensor_add(q2v, q1v[:, :, 0:16], q1v[:, :, 16:32])
            q3 = npool.tile([P, HEADS * 8], BF16, tag="q3")
            q3v = q3[:].rearrange("p (h d) -> p h d", d=8)
            nc.vector.tensor_add(q3v, q2v[:, :, 0:8], q2v[:, :, 8:16])
            q4 = npool.tile([P, HEADS * 4], BF16, tag="q4")
            q4v = q4[:].rearrange("p (h d) -> p h d", d=4)
            nc.vector.tensor_add(q4v, q3v[:, :, 0:4], q3v[:, :, 4:8])
            q5 = npool.tile([P, HEADS * 2], FP32, tag="q5")
            q5v = q5[:].rearrange("p (h d) -> p h d", d=2)
            nc.vector.tensor_add(q5v, q4v[:, :, 0:2], q4v[:, :, 2:4])
            rq = npool.tile([P, HEADS], FP32, tag="rq")
            nc.vector.tensor_add(rq[:], q5v[:, :, 0], q5v[:, :, 1])
            rqi = npool.tile([P, HEADS], FP32, tag="rqi")
            nc.vector.reciprocal_approx_fast(rqi[:], rq[:])
            s["rqi"] = rqi

        def emit_projV(i):
            mark(f"projV({i})")
            s = st[i]
            vt2 = evpool.tile([P, DIMS], BF16, tag="vt2", name=f"vt{i}")
            proj(i, xts[i], "v", vt2, None)
            s["vt2"] = vt2
            vd = dscr.tile([P, DIMS], BF16, tag="vscr", name=f"vd{i}")
            nc.sync.dma_start(vd[:], vt2[:])
            s["vd"] = vd

        def extract(j, src_t, tag):
            # per-head PE transposes -> slab [64, (h,t)] cols h*128+t
            slab = slabpool.tile([HD, P * HEADS], BF16, tag=tag,
                                 name=f"{tag}{j}")
            for b2 in range(2):
                ep = ps_ep.tile([HD, 8 * P], BF16, tag="ep", bufs=1,
                                name=f"ep{j}_{tag}{b2}")
                for hh in range(8):
                    h = 8 * b2 + hh
                    nc.tensor.transpose(
                        ep[:, hh * P:(hh + 1) * P],
                        src_t[:, h * HD:(h + 1) * HD], ident[:])
                if tag == "ks" or b2 == 0:
                    nc.vector.tensor_scalar_mul(
                        slab[:, b2 * 8 * P:(b2 + 1) * 8 * P], ep[:], 1.0)
                else:
                    nc.scalar.copy(slab[:, b2 * 8 * P:(b2 + 1) * 8 * P], ep[:])
            return slab

        def emit_vs2load(j):
            # VS2 gather from DRAM scratch: vs2[h', t*64+e] = V[t, h', e]
            # vscr row t (permuted wv: col e*16+h'): addr = t*1024 + e*16 + h'
            s = st[j]
            vs2 = vspool.tile([HEADS, P * HD], BF16, tag="vs2",
                              name=f"vs2{j}")
            srcv = s["vd"][:].rearrange("t (h e) -> h t e", e=HD)
            dstv = vs2[:].rearrange("h (t e) -> h t e", e=HD)
            # 4-way token split so each mm2 psum bank can start as soon as
            # its quarter of V has landed
            for q4 in range(4):
                ts4 = slice(q4 * 32, (q4 + 1) * 32)
                nc.sync.dma_start(dstv[:, ts4, :], srcv[:, ts4, :])
            s["vs2"] = vs2

        for i in range(N_TILES + 4):
            if i + 2 < N_TILES and (i + 2) not in xts:
                xts[i + 2] = load_x(i + 2)
            if 0 <= i - 1 < N_TILES and "vt2" in st.get(i - 1, {}):
                emit_vs2load(i - 1)
            if 0 <= i - 4 < N_TILES:
                emit_post(i - 4)
            if 0 <= i - 3 < N_TILES:
                emit_mm2_post(i - 3)
            if i < N_TILES:
                emit_projK(i)
            if 0 <= i - 2 < N_TILES:
                emit_mm1(i - 2)
            if i < N_TILES:
                emit_projQ(i)
                emit_projV(i)

    nc.compile()
    return nc


def _split8(a):
    hi = a.astype(ml_dtypes.float8_e4m3)
    lo = (a - hi.astype(np.float32)).astype(ml_dtypes.float8_e4m3)
    return hi, lo


def kernel(input_seq_embs, W_Q, W_K, W_V):
    x = np.asarray(input_seq_embs, dtype=np.float32).reshape(TOKENS, DIMS)
    # weights scaled by 32 so the fp8 residual stays in range; evictions
    # rescale by 1/32
    wq8, wqr = _split8(np.ascontiguousarray(np.asarray(W_Q, np.float32).T) * WSCALE)
    wk8, wkr = _split8(np.ascontiguousarray(np.asarray(W_K, np.float32).T) * WSCALE)
    wv8, wvr = _split8(np.ascontiguousarray(np.asarray(W_V, np.float32).T) * WSCALE)

    ident = np.eye(P, dtype=ml_dtypes.bfloat16)
    if "nc" not in _COMPILED:
        _COMPILED["nc"] = _build_kernel()
    nc = _COMPILED["nc"]

    in_maps = []
    for c in range(N_CORES):
        shard = x[c * TOK_PER_CORE:(c + 1) * TOK_PER_CORE]
        # pre-tiled transpose: row it*128+p, col c*128+t = x[it*128+t, c*128+p]
        x4 = shard.reshape(N_TILES, P, 8, P)              # [it, t, c, p]
        xt = np.ascontiguousarray(x4.transpose(0, 3, 2, 1)).reshape(
            TOK_PER_CORE, DIMS)
        x8, xr = _split8(xt)
        in_maps.append({"x8": x8, "xr": xr, "wq8": wq8, "wqr": wqr,
                        "wk8": wk8, "wkr": wkr, "wv8": wv8, "wvr": wvr,
                        "ident": ident})

    import os
    trace = bool(int(os.environ.get("KERNEL_PROFILE", "0")))
    kw = {}
    if trace:
        kw = dict(trace=True, tmpdir=os.environ.get("KERNEL_TRACE_DIR") or None)
    res = run_bass_kernel_spmd(nc, in_maps, list(range(N_CORES)), **kw)
    if trace:
        print(f"HW exec time: {res.exec_time_ns} ns")
        _COMPILED["last_result"] = res
    outs = [np.asarray(res.results[c]["out"], dtype=np.float32)
            for c in range(N_CORES)]
    return np.concatenate(outs, axis=0).reshape(B, L, DIMS)
